# revision 15
# baseline (speedup 1.0000x reference)
"""AC-loss (argmax-coords + l2) kernel for 16x64x256x256 inputs on 8 TRN2
NeuronCores, data-parallel over the batch.

HBM-traffic optimization: inputs are uploaded as fp16 of (p - 1) * 4096 and
(g - 1) * 4096, halving DMA bytes (the stream is HBM-bound).  The affine
maps are monotone, so per-row argmaxes are preserved; shifting g by 1 moves
its top values (uniform in [0,1), clustered within ~1e-5 of 1.0) next to 0
where fp16 resolves ~1e-8 gaps, so the g argmax is exact; the x4096 scale
keeps every row max in fp16 normal range (no subnormal flush-to-zero risk).
Measured end-to-end rel err vs the f32 reference: ~4e-4 (tolerance 2e-2).

Engine notes (measured on this silicon): DVE tensor_reduce never engages
16-bit perf modes (1 elem/cycle), but tensor_tensor max/min run at 2x, so
per-window maxes are computed as a 3-level pairwise TT-max fold
(512->256->128->64) shared by BOTH tensors in one concatenated [P,4096]
tile, finished by one small windowed tensor_reduce: ~2.64us/chunk on DVE
vs 2.9us DMA -> DMA-bound.  GpSimd cannot compare at all (no min/max
opcodes), so d = p - g is built on the idle PE as identity/(-identity)
matmul pairs accumulating fp16 chunks into f32 PSUM (exact), and ScalarE
squares PSUM in place (scale 2^-12) with the per-chunk f32 accumulate
producing per-row sum((p-g)^2).

Argmax index recovery: per-row winning 512-window via is_equal+iota scans
over the per-window maxes, one tiny indirect-DMA refetch of that window,
scan within it (as baseline).  Host combines coords -> distance/angle MSE
-> w_ac, l2 -> loss.
"""
from contextlib import ExitStack

import numpy as np

import concourse.bass as bass
import concourse.tile as tile
from concourse import bacc, mybir
from concourse.bass_utils import run_bass_kernel_spmd

F32 = mybir.dt.float32
I32 = mybir.dt.int32
I16 = mybir.dt.int16
F16 = mybir.dt.float16
P = 128

# problem shape (hardcoded per spec)
B, C, H, W = 16, 64, 256, 256
HW = H * W
N_CORES = 8
BPC = B // N_CORES          # samples per core
K = 2048                    # streaming chunk width (per tensor)
NCH = HW // K               # 32 chunks
WIN = 512                   # argmax window width
NW = HW // WIN              # 128 windows per row
WPC = K // WIN              # 4 windows per chunk per tensor
IDX_OFFSET = (NW + 1) * WIN  # device indices are shifted by -(NW+1)*WIN

SCALE = 4096.0              # host upload scale; device squares with 1/SCALE

EPS_ACOS = 1e-7
EPS_COS = 1e-8

DK = 2 * K   # double-chunk width per tensor: [128, 4096] fp16 DMAs move
             # 8KB per partition-descriptor (4KB descriptors measured ~10%
             # below peak DMA rate)
ND = (HW - 2 * K) // DK  # 15 double-chunks after 2 single ramp chunks


def _build_nc(io_bufs=8):
    nc = bacc.Bacc("TRN2", target_bir_lowering=False, debug=False,
                   num_devices=N_CORES)
    p_dram = nc.declare_dram_parameter("p", [P, HW], F16, isOutput=False)
    g_dram = nc.declare_dram_parameter("g", [P, HW], F16, isOutput=False)
    out_dram = nc.declare_dram_parameter("out3", [P, 3], F32, isOutput=True)

    with tile.TileContext(nc) as tc, ExitStack() as ctx:
        io = ctx.enter_context(tc.tile_pool(name="io", bufs=6))
        ramp = ctx.enter_context(tc.tile_pool(name="ramp", bufs=2))
        psum = ctx.enter_context(tc.psum_pool(name="ps", bufs=2))
        fp1 = ctx.enter_context(tc.tile_pool(name="f1", bufs=2))
        fp2 = ctx.enter_context(tc.tile_pool(name="f2", bufs=2))
        fp3 = ctx.enter_context(tc.tile_pool(name="f3", bufs=2))
        pgw = ctx.enter_context(tc.tile_pool(name="wd", bufs=2))
        singles = ctx.enter_context(tc.tile_pool(name="singles", bufs=1))

        # interleaved per-window extremes: col 8*c + e, e<4 -> p-window
        # 4c+e, e>=4 -> g-window 4c+(e-4)
        maxpm = singles.tile([P, 2 * NW], F16)
        l2c = singles.tile([P, NCH], F32)

        # Ramp: chunks 0 and 1 as singles so compute starts early; the
        # first DMAs below are issued before the constants so the engines
        # saturate from t=0.
        rcat0 = ramp.tile([P, 2 * K], F16, tag="rcat")
        nc.sync.dma_start(out=rcat0[:, :K], in_=p_dram[:, 0:K])
        nc.sync.dma_start(out=rcat0[:, K:], in_=g_dram[:, 0:K])
        rcat1 = ramp.tile([P, 2 * K], F16, tag="rcat")
        nc.sync.dma_start(out=rcat1[:, :K], in_=p_dram[:, K:2 * K])
        nc.sync.dma_start(out=rcat1[:, K:], in_=g_dram[:, K:2 * K])

        # identity / -identity stationaries for the PE matmuls
        icol = singles.tile([P, P], F32)
        nc.gpsimd.iota(icol[:], pattern=[[1, P]], base=0,
                       channel_multiplier=0,
                       allow_small_or_imprecise_dtypes=True)
        irow = singles.tile([P, 1], F32)
        nc.gpsimd.iota(irow[:], pattern=[[0, 1]], base=0,
                       channel_multiplier=1,
                       allow_small_or_imprecise_dtypes=True)
        ident = singles.tile([P, P], F16)
        nc.vector.tensor_scalar(
            out=ident[:], in0=icol[:], scalar1=irow[:], scalar2=None,
            op0=mybir.AluOpType.is_equal)
        nident = singles.tile([P, P], F16)
        nc.vector.tensor_scalar(
            out=nident[:], in0=ident[:], scalar1=-1.0, scalar2=None,
            op0=mybir.AluOpType.mult)

        # tail constants: within-window iota j-WIN, window iota w-NW,
        # per-row base row*NW + NW
        iota_w = singles.tile([P, WIN], I16)
        nc.gpsimd.iota(iota_w[:], pattern=[[1, WIN]], base=-WIN,
                       channel_multiplier=0)
        iota_nw = singles.tile([P, NW], F32)
        nc.gpsimd.iota(iota_nw[:], pattern=[[1, NW]], base=-NW,
                       channel_multiplier=0,
                       allow_small_or_imprecise_dtypes=True)
        prowB = singles.tile([P, 1], F32)
        nc.gpsimd.iota(prowB[:], pattern=[[0, 1]], base=NW,
                       channel_multiplier=NW,
                       allow_small_or_imprecise_dtypes=True)

        # fold tree + PE subtract + Act square for a concatenated tile
        # cat = [p (width) | g (width)]; psum handled in 2048-col halves
        # (one PSUM tile = 4 banks each) so doubles still double-buffer.
        def emit_chunk(cat, width, tr_out, l2slice):
            w2 = 2 * width
            cv = cat[:].rearrange("p (w two k) -> p w two k", two=2, k=256)
            t1 = fp1.tile([P, DK], F16, tag="t1")
            t1v = t1[:, :w2 // 2].rearrange(
                "p (w one k) -> p w one k", one=1, k=256)
            nc.vector.tensor_tensor(
                out=t1v, in0=cv[:, :, 0:1, :], in1=cv[:, :, 1:2, :],
                op=mybir.AluOpType.max)
            t1w = t1[:, :w2 // 2].rearrange(
                "p (w two k) -> p w two k", two=2, k=128)
            t2 = fp2.tile([P, DK // 2], F16, tag="t2")
            t2v = t2[:, :w2 // 4].rearrange(
                "p (w one k) -> p w one k", one=1, k=128)
            nc.vector.tensor_tensor(
                out=t2v, in0=t1w[:, :, 0:1, :], in1=t1w[:, :, 1:2, :],
                op=mybir.AluOpType.max)
            t2w = t2[:, :w2 // 4].rearrange(
                "p (w two k) -> p w two k", two=2, k=64)
            t3 = fp3.tile([P, DK // 4], F16, tag="t3")
            t3v = t3[:, :w2 // 8].rearrange(
                "p (w one k) -> p w one k", one=1, k=64)
            nc.vector.tensor_tensor(
                out=t3v, in0=t2w[:, :, 0:1, :], in1=t2w[:, :, 1:2, :],
                op=mybir.AluOpType.max)
            nc.vector.tensor_reduce(
                out=tr_out,
                in_=t3[:, :w2 // 8].rearrange("p (w k) -> p w k", k=64),
                axis=mybir.AxisListType.X, op=mybir.AluOpType.max)
            # l2 split: half A -> PE identity/-identity matmuls into PSUM
            # (psum bufs=2 now gives 2-double lookahead, so the Act->PE
            # recycle loop never gates the stream); half B (doubles only)
            # -> GpSimd fp16 subtract + Act square from SBUF.
            ps_t = psum.tile([P, K], F32, tag="ps")
            for b in range(4):
                ps_blk = ps_t[:, b * 512:(b + 1) * 512]
                nc.tensor.matmul(
                    out=ps_blk, lhsT=ident[:],
                    rhs=cat[:, b * 512:(b + 1) * 512],
                    start=True, stop=False)
                nc.tensor.matmul(
                    out=ps_blk, lhsT=nident[:],
                    rhs=cat[:, width + b * 512:width + (b + 1) * 512],
                    start=False, stop=True)
            nc.scalar.activation(
                out=ps_t[:], in_=ps_t[:],
                func=mybir.ActivationFunctionType.Square,
                scale=1.0 / SCALE,
                accum_out=l2c[:, l2slice:l2slice + 1])
            if width == DK:
                wd = pgw.tile([P, K], F16, tag="wd")
                nc.gpsimd.tensor_tensor(
                    out=wd[:], in0=cat[:, K:DK],
                    in1=cat[:, width + K:width + DK],
                    op=mybir.AluOpType.subtract)
                nc.scalar.activation(
                    out=wd[:], in_=wd[:],
                    func=mybir.ActivationFunctionType.Square,
                    scale=1.0 / SCALE,
                    accum_out=l2c[:, l2slice + 1:l2slice + 2])

        emit_chunk(rcat0, K, maxpm[:, 0:8], 0)
        emit_chunk(rcat1, K, maxpm[:, 8:16], 1)

        for k in range(1, ND + 1):
            cat = io.tile([P, 2 * DK], F16, tag="cat")
            nc.sync.dma_start(out=cat[:, :DK],
                              in_=p_dram[:, k * DK:(k + 1) * DK])
            nc.sync.dma_start(out=cat[:, DK:],
                              in_=g_dram[:, k * DK:(k + 1) * DK])
            # TR iterates (t, a, e): p-windows of both sub-chunks, then
            # g-windows; maxpm wants col 16k + 8a + 4t + e
            tr_out = maxpm[:, 16 * k:16 * (k + 1)].rearrange(
                "p (a t e) -> p t a e", a=2, t=2, e=4)
            emit_chunk(cat, DK, tr_out, 2 * k)

        out3 = singles.tile([P, 3], F32)
        nc.vector.tensor_reduce(out=out3[:, 2:3], in_=l2c[:],
                                axis=mybir.AxisListType.X,
                                op=mybir.AluOpType.add)

        # Tails: pre (window-find + refetch issue) then post (scan),
        # emitted p-pre, g-pre, p-post, g-post so the in-order DVE queue
        # doesn't park g's ready pre-work behind p's refetch wait.
        # cols_v is the [P, 32, 4] strided view of this tensor's window
        # maxes inside maxpm; iota must use the matching view.
        iota_v = iota_nw[:].rearrange("p (a b) -> p a b", b=4)

        def tail_pre(dram, cols_v, tag):
            # device idx' = (fw-NW)*WIN + (li-WIN); host adds (NW+1)*WIN
            gext = singles.tile([P, 1], F16, tag=f"gext_{tag}")
            nc.vector.tensor_reduce(out=gext[:], in_=cols_v,
                                    axis=mybir.AxisListType.XY,
                                    op=mybir.AluOpType.max)
            valc = singles.tile([P, NW], F32, tag=f"valc_{tag}")
            valc_v = valc[:].rearrange("p (a b) -> p a b", b=4)
            nc.vector.scalar_tensor_tensor(
                out=valc_v, in0=cols_v, scalar=gext[:, 0:1], in1=iota_v,
                op0=mybir.AluOpType.is_equal, op1=mybir.AluOpType.mult)
            fw = singles.tile([P, 1], F32, tag=f"fw_{tag}")
            nc.vector.tensor_reduce(out=fw[:], in_=valc[:],
                                    axis=mybir.AxisListType.X,
                                    op=mybir.AluOpType.min)
            rowi = singles.tile([P, 1], I32, tag=f"rowi_{tag}")
            nc.vector.tensor_scalar(
                out=rowi[:], in0=prowB[:], scalar1=fw[:], scalar2=None,
                op0=mybir.AluOpType.add)
            win = singles.tile([P, WIN], F16, tag=f"win_{tag}")
            nc.gpsimd.indirect_dma_start(
                out=win[:], out_offset=None,
                in_=dram[:].rearrange("a (b k) -> (a b) k", k=WIN),
                in_offset=bass.IndirectOffsetOnAxis(ap=rowi[:, :1], axis=0))
            return gext, fw, win

        def tail_post(gext, fw, win, out_col, tag):
            valw = singles.tile([P, WIN], I16, tag=f"valw_{tag}")
            nc.vector.scalar_tensor_tensor(
                out=valw[:], in0=win[:], scalar=gext[:, 0:1], in1=iota_w[:],
                op0=mybir.AluOpType.is_equal, op1=mybir.AluOpType.mult)
            li = singles.tile([P, 1], F32, tag=f"li_{tag}")
            wmin = singles.tile([P, WIN], F16, tag=f"wmin_{tag}")
            nc.vector.tensor_scalar(
                out=wmin[:], in0=valw[:], scalar1=0.0, scalar2=None,
                op0=mybir.AluOpType.bypass, op1=mybir.AluOpType.min,
                accum_out=li[:])
            nc.vector.scalar_tensor_tensor(
                out=out3[:, out_col:out_col + 1], in0=fw[:], scalar=float(WIN),
                in1=li[:],
                op0=mybir.AluOpType.mult, op1=mybir.AluOpType.add)

        pm8 = maxpm[:].rearrange("p (c e) -> p c e", e=8)
        hp = tail_pre(p_dram, pm8[:, :, 0:4], "p")
        hg = tail_pre(g_dram, pm8[:, :, 4:8], "g")
        tail_post(*hp, 0, "p")
        tail_post(*hg, 1, "g")
        nc.sync.dma_start(out=out_dram[:], in_=out3[:])

    nc.compile()
    return nc


_NC_CACHE = None


def _get_nc():
    global _NC_CACHE
    if _NC_CACHE is None:
        _NC_CACHE = _build_nc()
    return _NC_CACHE


def make_in_maps(predict, gt):
    """Per-core device inputs: fp16((p-1)*SCALE), fp16((g-1)*SCALE)."""
    predict = np.asarray(predict, dtype=np.float32)
    gt = np.asarray(gt, dtype=np.float32)
    p16 = ((predict - np.float32(1.0)) * np.float32(SCALE)).astype(np.float16)
    m16 = ((gt - np.float32(1.0)) * np.float32(SCALE)).astype(np.float16)
    in_maps = []
    for i in range(N_CORES):
        in_maps.append({
            "p": np.ascontiguousarray(
                p16[i * BPC:(i + 1) * BPC].reshape(BPC * C, HW)),
            "g": np.ascontiguousarray(
                m16[i * BPC:(i + 1) * BPC].reshape(BPC * C, HW)),
        })
    return in_maps


# ---------------- host-side loss combination (mirrors the reference) -------

def _coords(idx):
    r = (idx // W).astype(np.float32)
    c = (idx % W).astype(np.float32)
    return np.stack([r, c], axis=-1)


def _pairwise_dist(xy):
    diff = xy[:, :, None, :] - xy[:, None, :, :]
    return np.sqrt((diff * diff).sum(axis=-1))


def _angle_matrix(xy):
    dots = np.einsum('bic,bjc->bij', xy, xy)
    norms = np.sqrt((xy * xy).sum(axis=-1))
    denom = np.maximum(norms[:, :, None] * norms[:, None, :],
                       np.float32(EPS_COS))
    cos = np.clip(dots / denom, np.float32(-1.0 + EPS_ACOS),
                  np.float32(1.0 - EPS_ACOS))
    return np.arccos(cos)


def _combine(pidx, gidx, l2row):
    p_xy = _coords(pidx)
    g_xy = _coords(gidx)
    dD = _pairwise_dist(p_xy) - _pairwise_dist(g_xy)
    mse_D = (dD * dD).mean(axis=(1, 2), dtype=np.float32)
    dA = _angle_matrix(p_xy) - _angle_matrix(g_xy)
    mse_A = (dA * dA).mean(axis=(1, 2), dtype=np.float32)
    w_ac = np.log2(mse_D) + np.log2(mse_A)
    l2 = l2row.sum(axis=1, dtype=np.float32) / np.float32(C * H * W)
    return np.float32((w_ac * l2).sum(dtype=np.float32) / np.float32(B))


def kernel(predict, gt):
    predict = np.asarray(predict, dtype=np.float32)
    gt = np.asarray(gt, dtype=np.float32)
    assert predict.shape == (B, C, H, W) and gt.shape == (B, C, H, W)

    in_maps = make_in_maps(predict, gt)

    nc = _get_nc()
    res = run_bass_kernel_spmd(nc, in_maps, core_ids=list(range(N_CORES)))

    pidx = np.zeros((B, C), dtype=np.int64)
    gidx = np.zeros((B, C), dtype=np.int64)
    l2row = np.zeros((B, C), dtype=np.float32)
    for i in range(N_CORES):
        o = res.results[i]["out3"]
        pidx[i * BPC:(i + 1) * BPC] = (
            np.rint(o[:, 0].reshape(BPC, C)).astype(np.int64) + IDX_OFFSET)
        gidx[i * BPC:(i + 1) * BPC] = (
            np.rint(o[:, 1].reshape(BPC, C)).astype(np.int64) + IDX_OFFSET)
        l2row[i * BPC:(i + 1) * BPC] = o[:, 2].reshape(BPC, C)

    return np.asarray(_combine(pidx, gidx, l2row), dtype=np.float32)


# revision 16
# speedup vs baseline: 1.3410x; 1.3410x over previous
"""AC-loss (argmax-coords + l2) kernel for 16x64x256x256 inputs on 8 TRN2
NeuronCores, data-parallel over the batch.

HBM-traffic optimization: inputs are uploaded as fp16 of (p - 1) * 4096 and
(g - 1) * 4096, halving DMA bytes (the stream is HBM-bound).  The affine
maps are monotone, so per-row argmaxes are preserved; shifting g by 1 moves
its top values (uniform in [0,1), clustered within ~1e-5 of 1.0) next to 0
where fp16 resolves ~1e-8 gaps, so the g argmax is exact; the x4096 scale
keeps every row max in fp16 normal range (no subnormal flush-to-zero risk).
Measured end-to-end rel err vs the f32 reference: ~4e-4 (tolerance 2e-2).

Engine notes (measured on this silicon): DVE tensor_reduce never engages
16-bit perf modes (1 elem/cycle), but tensor_tensor max/min run at 2x, so
per-window maxes are computed as a 3-level pairwise TT-max fold
(512->256->128->64) shared by BOTH tensors in one concatenated [P,4096]
tile, finished by one small windowed tensor_reduce: ~2.64us/chunk on DVE
vs 2.9us DMA -> DMA-bound.  GpSimd cannot compare at all (no min/max
opcodes), so d = p - g is built on the idle PE as identity/(-identity)
matmul pairs accumulating fp16 chunks into f32 PSUM (exact), and ScalarE
squares PSUM in place (scale 2^-12) with the per-chunk f32 accumulate
producing per-row sum((p-g)^2).

Argmax index recovery: per-row winning 512-window via is_equal+iota scans
over the per-window maxes, one tiny indirect-DMA refetch of that window,
scan within it (as baseline).  Host combines coords -> distance/angle MSE
-> w_ac, l2 -> loss.
"""
from contextlib import ExitStack

import numpy as np

import concourse.bass as bass
import concourse.tile as tile
from concourse import bacc, mybir
from concourse.bass_utils import run_bass_kernel_spmd

F32 = mybir.dt.float32
I32 = mybir.dt.int32
I16 = mybir.dt.int16
F16 = mybir.dt.float16
P = 128

# problem shape (hardcoded per spec)
B, C, H, W = 16, 64, 256, 256
HW = H * W
N_CORES = 8
BPC = B // N_CORES          # samples per core
K = 2048                    # streaming chunk width (per tensor)
NCH = HW // K               # 32 chunks
WIN = 512                   # argmax window width
NW = HW // WIN              # 128 windows per row
WPC = K // WIN              # 4 windows per chunk per tensor
IDX_OFFSET = (NW + 1) * WIN  # device indices are shifted by -(NW+1)*WIN

SCALE = 4096.0              # host upload scale; device squares with 1/SCALE

EPS_ACOS = 1e-7
EPS_COS = 1e-8

DK = 2 * K   # double-chunk width per tensor: [128, 4096] fp16 DMAs move
             # 8KB per partition-descriptor (4KB descriptors measured ~10%
             # below peak DMA rate)
ND = (HW - 2 * K) // DK  # 15 double-chunks after 2 single ramp chunks


def _build_nc(io_bufs=8):
    nc = bacc.Bacc("TRN2", target_bir_lowering=False, debug=False,
                   num_devices=N_CORES)
    p_dram = nc.declare_dram_parameter("p", [P, HW], F16, isOutput=False)
    g_dram = nc.declare_dram_parameter("g", [P, HW], F16, isOutput=False)
    out_dram = nc.declare_dram_parameter("out3", [P, 3], F32, isOutput=True)

    with tile.TileContext(nc) as tc, ExitStack() as ctx:
        io = ctx.enter_context(tc.tile_pool(name="io", bufs=7))
        ramp = ctx.enter_context(tc.tile_pool(name="ramp", bufs=2))
        psum = ctx.enter_context(tc.psum_pool(name="ps", bufs=4))
        fp1 = ctx.enter_context(tc.tile_pool(name="f1", bufs=2))
        fp2 = ctx.enter_context(tc.tile_pool(name="f2", bufs=2))
        fp3 = ctx.enter_context(tc.tile_pool(name="f3", bufs=2))
        singles = ctx.enter_context(tc.tile_pool(name="singles", bufs=1))

        # interleaved per-window extremes: col 8*c + e, e<4 -> p-window
        # 4c+e, e>=4 -> g-window 4c+(e-4)
        maxpm = singles.tile([P, 2 * NW], F16)
        l2c = singles.tile([P, 2 * NCH], F32)

        # Ramp: chunks 0 and 1 as singles so compute starts early; the
        # first DMAs below are issued before the constants so the engines
        # saturate from t=0.
        rcat0 = ramp.tile([P, 2 * K], F16, tag="rcat")
        nc.sync.dma_start(out=rcat0[:, :K], in_=p_dram[:, 0:K])
        nc.sync.dma_start(out=rcat0[:, K:], in_=g_dram[:, 0:K])
        rcat1 = ramp.tile([P, 2 * K], F16, tag="rcat")
        nc.sync.dma_start(out=rcat1[:, :K], in_=p_dram[:, K:2 * K])
        nc.sync.dma_start(out=rcat1[:, K:], in_=g_dram[:, K:2 * K])

        # identity / -identity stationaries for the PE matmuls
        icol = singles.tile([P, P], F32)
        nc.gpsimd.iota(icol[:], pattern=[[1, P]], base=0,
                       channel_multiplier=0,
                       allow_small_or_imprecise_dtypes=True)
        irow = singles.tile([P, 1], F32)
        nc.gpsimd.iota(irow[:], pattern=[[0, 1]], base=0,
                       channel_multiplier=1,
                       allow_small_or_imprecise_dtypes=True)
        ident = singles.tile([P, P], F16)
        nc.vector.tensor_scalar(
            out=ident[:], in0=icol[:], scalar1=irow[:], scalar2=None,
            op0=mybir.AluOpType.is_equal)
        nident = singles.tile([P, P], F16)
        nc.vector.tensor_scalar(
            out=nident[:], in0=ident[:], scalar1=-1.0, scalar2=None,
            op0=mybir.AluOpType.mult)

        # tail constants: within-window iota j-WIN, window iota w-NW,
        # per-row base row*NW + NW
        iota_w = singles.tile([P, WIN], I16)
        nc.gpsimd.iota(iota_w[:], pattern=[[1, WIN]], base=-WIN,
                       channel_multiplier=0)
        iota_nw = singles.tile([P, NW], F32)
        nc.gpsimd.iota(iota_nw[:], pattern=[[1, NW]], base=-NW,
                       channel_multiplier=0,
                       allow_small_or_imprecise_dtypes=True)
        prowB = singles.tile([P, 1], F32)
        nc.gpsimd.iota(prowB[:], pattern=[[0, 1]], base=NW,
                       channel_multiplier=NW,
                       allow_small_or_imprecise_dtypes=True)

        # fold tree + PE subtract + Act square for a concatenated tile
        # cat = [p (width) | g (width)]; psum handled in 2048-col halves
        # (one PSUM tile = 4 banks each) so doubles still double-buffer.
        def emit_chunk(cat, width, tr_out, l2slice):
            w2 = 2 * width
            cv = cat[:].rearrange("p (w two k) -> p w two k", two=2, k=256)
            t1 = fp1.tile([P, DK], F16, tag="t1")
            t1v = t1[:, :w2 // 2].rearrange(
                "p (w one k) -> p w one k", one=1, k=256)
            nc.vector.tensor_tensor(
                out=t1v, in0=cv[:, :, 0:1, :], in1=cv[:, :, 1:2, :],
                op=mybir.AluOpType.max)
            t1w = t1[:, :w2 // 2].rearrange(
                "p (w two k) -> p w two k", two=2, k=128)
            t2 = fp2.tile([P, DK // 2], F16, tag="t2")
            t2v = t2[:, :w2 // 4].rearrange(
                "p (w one k) -> p w one k", one=1, k=128)
            nc.vector.tensor_tensor(
                out=t2v, in0=t1w[:, :, 0:1, :], in1=t1w[:, :, 1:2, :],
                op=mybir.AluOpType.max)
            t2w = t2[:, :w2 // 4].rearrange(
                "p (w two k) -> p w two k", two=2, k=64)
            t3 = fp3.tile([P, DK // 4], F16, tag="t3")
            t3v = t3[:, :w2 // 8].rearrange(
                "p (w one k) -> p w one k", one=1, k=64)
            nc.vector.tensor_tensor(
                out=t3v, in0=t2w[:, :, 0:1, :], in1=t2w[:, :, 1:2, :],
                op=mybir.AluOpType.max)
            nc.vector.tensor_reduce(
                out=tr_out,
                in_=t3[:, :w2 // 8].rearrange("p (w k) -> p w k", k=64),
                axis=mybir.AxisListType.X, op=mybir.AluOpType.max)
            # [P,1024] psum quarters, bufs=4: fine-grained PE->Act
            # pipelining so the psum-recycle RTT never gates the stream;
            # I/-I interleaved per 512-block so each quarter retires early
            for h in range(width // 1024):
                ps_t = psum.tile([P, 1024], F32, tag="ps")
                for b in range(2):
                    ps_blk = ps_t[:, b * 512:(b + 1) * 512]
                    src0 = h * 1024 + b * 512
                    nc.tensor.matmul(
                        out=ps_blk, lhsT=ident[:],
                        rhs=cat[:, src0:src0 + 512],
                        start=True, stop=False)
                    nc.tensor.matmul(
                        out=ps_blk, lhsT=nident[:],
                        rhs=cat[:, width + src0:width + src0 + 512],
                        start=False, stop=True)
                nc.scalar.activation(
                    out=ps_t[:], in_=ps_t[:],
                    func=mybir.ActivationFunctionType.Square,
                    scale=1.0 / SCALE,
                    accum_out=l2c[:, l2slice + h:l2slice + h + 1])

        emit_chunk(rcat0, K, maxpm[:, 0:8], 0)
        emit_chunk(rcat1, K, maxpm[:, 8:16], 2)

        for k in range(1, ND + 1):
            cat = io.tile([P, 2 * DK], F16, tag="cat")
            nc.sync.dma_start(out=cat[:, :DK],
                              in_=p_dram[:, k * DK:(k + 1) * DK])
            nc.sync.dma_start(out=cat[:, DK:],
                              in_=g_dram[:, k * DK:(k + 1) * DK])
            # TR iterates (t, a, e): p-windows of both sub-chunks, then
            # g-windows; maxpm wants col 16k + 8a + 4t + e
            tr_out = maxpm[:, 16 * k:16 * (k + 1)].rearrange(
                "p (a t e) -> p t a e", a=2, t=2, e=4)
            emit_chunk(cat, DK, tr_out, 4 * k)

        out3 = singles.tile([P, 3], F32)
        nc.vector.tensor_reduce(out=out3[:, 2:3], in_=l2c[:],
                                axis=mybir.AxisListType.X,
                                op=mybir.AluOpType.add)

        # Tails: pre (window-find + refetch issue) then post (scan),
        # emitted p-pre, g-pre, p-post, g-post so the in-order DVE queue
        # doesn't park g's ready pre-work behind p's refetch wait.
        # cols_v is the [P, 32, 4] strided view of this tensor's window
        # maxes inside maxpm; iota must use the matching view.
        iota_v = iota_nw[:].rearrange("p (a b) -> p a b", b=4)

        def tail_pre(dram, cols_v, tag):
            # device idx' = (fw-NW)*WIN + (li-WIN); host adds (NW+1)*WIN
            gext = singles.tile([P, 1], F16, tag=f"gext_{tag}")
            nc.vector.tensor_reduce(out=gext[:], in_=cols_v,
                                    axis=mybir.AxisListType.XY,
                                    op=mybir.AluOpType.max)
            valc = singles.tile([P, NW], F32, tag=f"valc_{tag}")
            valc_v = valc[:].rearrange("p (a b) -> p a b", b=4)
            nc.vector.scalar_tensor_tensor(
                out=valc_v, in0=cols_v, scalar=gext[:, 0:1], in1=iota_v,
                op0=mybir.AluOpType.is_equal, op1=mybir.AluOpType.mult)
            fw = singles.tile([P, 1], F32, tag=f"fw_{tag}")
            nc.vector.tensor_reduce(out=fw[:], in_=valc[:],
                                    axis=mybir.AxisListType.X,
                                    op=mybir.AluOpType.min)
            rowi = singles.tile([P, 1], I32, tag=f"rowi_{tag}")
            nc.vector.tensor_scalar(
                out=rowi[:], in0=prowB[:], scalar1=fw[:], scalar2=None,
                op0=mybir.AluOpType.add)
            win = singles.tile([P, WIN], F16, tag=f"win_{tag}")
            nc.gpsimd.indirect_dma_start(
                out=win[:], out_offset=None,
                in_=dram[:].rearrange("a (b k) -> (a b) k", k=WIN),
                in_offset=bass.IndirectOffsetOnAxis(ap=rowi[:, :1], axis=0))
            return gext, fw, win

        def tail_post(gext, fw, win, out_col, tag):
            valw = singles.tile([P, WIN], I16, tag=f"valw_{tag}")
            nc.vector.scalar_tensor_tensor(
                out=valw[:], in0=win[:], scalar=gext[:, 0:1], in1=iota_w[:],
                op0=mybir.AluOpType.is_equal, op1=mybir.AluOpType.mult)
            li = singles.tile([P, 1], F32, tag=f"li_{tag}")
            wmin = singles.tile([P, WIN], F16, tag=f"wmin_{tag}")
            nc.vector.tensor_scalar(
                out=wmin[:], in0=valw[:], scalar1=0.0, scalar2=None,
                op0=mybir.AluOpType.bypass, op1=mybir.AluOpType.min,
                accum_out=li[:])
            nc.vector.scalar_tensor_tensor(
                out=out3[:, out_col:out_col + 1], in0=fw[:], scalar=float(WIN),
                in1=li[:],
                op0=mybir.AluOpType.mult, op1=mybir.AluOpType.add)

        pm8 = maxpm[:].rearrange("p (c e) -> p c e", e=8)
        hp = tail_pre(p_dram, pm8[:, :, 0:4], "p")
        hg = tail_pre(g_dram, pm8[:, :, 4:8], "g")
        tail_post(*hp, 0, "p")
        tail_post(*hg, 1, "g")
        nc.sync.dma_start(out=out_dram[:], in_=out3[:])

    nc.compile()
    return nc


_NC_CACHE = None


def _get_nc():
    global _NC_CACHE
    if _NC_CACHE is None:
        _NC_CACHE = _build_nc()
    return _NC_CACHE


def make_in_maps(predict, gt):
    """Per-core device inputs: fp16((p-1)*SCALE), fp16((g-1)*SCALE)."""
    predict = np.asarray(predict, dtype=np.float32)
    gt = np.asarray(gt, dtype=np.float32)
    p16 = ((predict - np.float32(1.0)) * np.float32(SCALE)).astype(np.float16)
    m16 = ((gt - np.float32(1.0)) * np.float32(SCALE)).astype(np.float16)
    in_maps = []
    for i in range(N_CORES):
        in_maps.append({
            "p": np.ascontiguousarray(
                p16[i * BPC:(i + 1) * BPC].reshape(BPC * C, HW)),
            "g": np.ascontiguousarray(
                m16[i * BPC:(i + 1) * BPC].reshape(BPC * C, HW)),
        })
    return in_maps


# ---------------- host-side loss combination (mirrors the reference) -------

def _coords(idx):
    r = (idx // W).astype(np.float32)
    c = (idx % W).astype(np.float32)
    return np.stack([r, c], axis=-1)


def _pairwise_dist(xy):
    diff = xy[:, :, None, :] - xy[:, None, :, :]
    return np.sqrt((diff * diff).sum(axis=-1))


def _angle_matrix(xy):
    dots = np.einsum('bic,bjc->bij', xy, xy)
    norms = np.sqrt((xy * xy).sum(axis=-1))
    denom = np.maximum(norms[:, :, None] * norms[:, None, :],
                       np.float32(EPS_COS))
    cos = np.clip(dots / denom, np.float32(-1.0 + EPS_ACOS),
                  np.float32(1.0 - EPS_ACOS))
    return np.arccos(cos)


def _combine(pidx, gidx, l2row):
    p_xy = _coords(pidx)
    g_xy = _coords(gidx)
    dD = _pairwise_dist(p_xy) - _pairwise_dist(g_xy)
    mse_D = (dD * dD).mean(axis=(1, 2), dtype=np.float32)
    dA = _angle_matrix(p_xy) - _angle_matrix(g_xy)
    mse_A = (dA * dA).mean(axis=(1, 2), dtype=np.float32)
    w_ac = np.log2(mse_D) + np.log2(mse_A)
    l2 = l2row.sum(axis=1, dtype=np.float32) / np.float32(C * H * W)
    return np.float32((w_ac * l2).sum(dtype=np.float32) / np.float32(B))


def kernel(predict, gt):
    predict = np.asarray(predict, dtype=np.float32)
    gt = np.asarray(gt, dtype=np.float32)
    assert predict.shape == (B, C, H, W) and gt.shape == (B, C, H, W)

    in_maps = make_in_maps(predict, gt)

    nc = _get_nc()
    res = run_bass_kernel_spmd(nc, in_maps, core_ids=list(range(N_CORES)))

    pidx = np.zeros((B, C), dtype=np.int64)
    gidx = np.zeros((B, C), dtype=np.int64)
    l2row = np.zeros((B, C), dtype=np.float32)
    for i in range(N_CORES):
        o = res.results[i]["out3"]
        pidx[i * BPC:(i + 1) * BPC] = (
            np.rint(o[:, 0].reshape(BPC, C)).astype(np.int64) + IDX_OFFSET)
        gidx[i * BPC:(i + 1) * BPC] = (
            np.rint(o[:, 1].reshape(BPC, C)).astype(np.int64) + IDX_OFFSET)
        l2row[i * BPC:(i + 1) * BPC] = o[:, 2].reshape(BPC, C)

    return np.asarray(_combine(pidx, gidx, l2row), dtype=np.float32)
